# revision 9
# baseline (speedup 1.0000x reference)
"""Causal self-attention (B=4, T=2048, C=1024, H=16) on 8 trn2 NeuronCores.

Sharding: core i = 2*b + g handles batch b (of 4) and head-group g (of 2,
8 heads each).  Inside each core:
  phase 1: QKV projection.  x^T (pre-transposed on host, [C, T]) resident in
           SBUF; Q^T/K^T computed head-dim-major [512, T]; V token-major
           [T, 512] with a ones column appended per head ([V_h | 1]).
  phase 2: attention per (head, q-chunk of 512).  Scores computed transposed
           (S^T[k, q] = K Q^T) so the softmax axis (k) is the partition dim
           of the PV matmul; exp on ScalarE (no max subtraction needed at
           these magnitudes); causal handled by a triangular mask on diagonal
           128x128 blocks + zeroing below-diagonal blocks; PV produces
           y^T[d, q] with row 64 = softmax denominator (from the ones col);
           normalization = reciprocal + ones-matmul broadcast + multiply.
  phase 3: projection y^T @ W_proj rows -> per-core partial [T, C].
Host sums the two partials per batch and adds b_proj.
"""

import os
import sys

for _p in ("/opt/trn_rl_repo", "/opt/pypackages"):
    if _p not in sys.path and os.path.isdir(_p):
        sys.path.append(_p)

import numpy as np

import concourse.bass as bass
import concourse.bacc as bacc
import concourse.mybir as mybir
from concourse.tile import TileContext
from concourse.bass_utils import run_bass_kernel_spmd

F32 = mybir.dt.float32
# Matmul operand dtype: float32r streams fp32 at 1 cycle/row (vs 4 for plain
# fp32) when the moving free dim is >= 256, at ~tf32 precision (~1.5e-4 rel
# err measured on HW for K=128).  Every producer of an fp32r matmul operand
# must itself write float32r (BIR verifier rule), so operand tiles and their
# DRAM sources are declared float32r.  Flip to F32 if accuracy fails.
MMD = mybir.dt.float32r

T = 2048          # tokens
C = 1024          # embed dim
D = 64            # head dim
HL = 8            # heads per core
CL = HL * D       # 512 local channels
FT = C // 128     # 8 feature tiles
NRT = CL // 128   # 4 row tiles of Q^T/K^T/y^T
NTT = T // 128    # 16 token tiles
QCH = 512         # q chunk
NCH = T // QCH    # 4 chunks
SCALE = 1.0 / 8.0  # 1/sqrt(D)


def _qkv_phase(nc, tc, xt, wq, wk, wv, qt_sb, kt_sb, v_sb, bq_sb, bk_sb,
               bv_sb, ps_m_pool):
    with tc.tile_pool(name="xtw", bufs=1) as xtw:
        xt_sb = xtw.tile([128, FT, T], MMD, tag="xt")
        xt_r = xt[:].rearrange("(f p) n -> p f n", p=128)
        for ft in range(FT):
            nc.sync.dma_start(out=xt_sb[:, ft, :], in_=xt_r[:, ft, :])

        for w_dram, dest, bias in ((wq, qt_sb, bq_sb), (wk, kt_sb, bk_sb)):
            w_sb = xtw.tile([128, FT, CL], MMD, tag="w")
            nc.sync.dma_start(
                out=w_sb, in_=w_dram[:].rearrange("(f p) n -> p f n", p=128))
            for rt in range(NRT):
                for nt in range(NCH):
                    ps = ps_m_pool.tile([128, 512], F32, tag="mm")
                    for ft in range(FT):
                        nc.tensor.matmul(
                            ps,
                            lhsT=w_sb[:, ft, rt * 128:(rt + 1) * 128],
                            rhs=xt_sb[:, ft, nt * 512:(nt + 1) * 512],
                            start=(ft == 0), stop=(ft == FT - 1),
                        )
                    nc.vector.tensor_scalar_add(
                        dest[:, rt, nt * 512:(nt + 1) * 512],
                        ps, bias[:, rt:rt + 1])

        wv_sb = xtw.tile([128, FT, CL], MMD, tag="w")
        nc.sync.dma_start(
            out=wv_sb, in_=wv[:].rearrange("(f p) n -> p f n", p=128))
        bv_r = bv_sb.rearrange("p (h d) -> p h d", d=D)
        for tt in range(NTT):
            ps = ps_m_pool.tile([128, 512], F32, tag="mm")
            for ft in range(FT):
                nc.tensor.matmul(
                    ps,
                    lhsT=xt_sb[:, ft, tt * 128:(tt + 1) * 128],
                    rhs=wv_sb[:, ft, :],
                    start=(ft == 0), stop=(ft == FT - 1),
                )
            nc.vector.tensor_add(
                v_sb[:, tt, :, 0:D],
                ps.rearrange("p (h d) -> p h d", d=D), bv_r)


def build_nc():
    nc = bacc.Bacc()
    xt = nc.declare_dram_parameter("xt", [C, T], MMD, isOutput=False)
    wq = nc.declare_dram_parameter("wq", [C, CL], MMD, isOutput=False)
    wk = nc.declare_dram_parameter("wk", [C, CL], MMD, isOutput=False)
    wv = nc.declare_dram_parameter("wv", [C, CL], MMD, isOutput=False)
    wp = nc.declare_dram_parameter("wp", [CL, C], MMD, isOutput=False)
    bq = nc.declare_dram_parameter("bq", [CL], F32, isOutput=False)
    bk = nc.declare_dram_parameter("bk", [CL], F32, isOutput=False)
    bv = nc.declare_dram_parameter("bv", [CL], F32, isOutput=False)
    out = nc.declare_dram_parameter("out", [T, C], F32, isOutput=True)

    with TileContext(nc) as tc:
        with (
            tc.tile_pool(name="consts", bufs=1) as consts,
            tc.tile_pool(name="qkv", bufs=1) as qkvp,
            tc.tile_pool(name="ps_s", bufs=2, space="PSUM") as ps_s_pool,
            tc.tile_pool(name="ps_y", bufs=2, space="PSUM") as ps_y_pool,
            tc.tile_pool(name="ps_m", bufs=2, space="PSUM") as ps_m_pool,
        ):
            # ---- constants -------------------------------------------------
            # diag_mask[p, c] = 1.0 if p <= c else 0.0  (valid = k <= q)
            diag = consts.tile([128, 128], F32)
            nc.gpsimd.memset(diag, 1.0)
            nc.gpsimd.affine_select(
                out=diag, in_=diag,
                compare_op=mybir.AluOpType.is_ge,
                fill=0.0, base=0,
                pattern=[[1, 128]], channel_multiplier=-1,
            )
            one_f32 = consts.tile([128, 128], F32, tag="one_f32")
            nc.vector.memset(one_f32, 1.0)
            zero_f32 = consts.tile([128, 384], F32, tag="zero_f32")
            nc.vector.memset(zero_f32, 0.0)
            ones = consts.tile([128, D], MMD)
            nc.vector.tensor_copy(ones, one_f32[:, 0:D])

            bq_sb = consts.tile([128, NRT], F32)
            bk_sb = consts.tile([128, NRT], F32)
            nc.sync.dma_start(out=bq_sb,
                              in_=bq[:].rearrange("(r p) -> p r", p=128))
            nc.sync.dma_start(out=bk_sb,
                              in_=bk[:].rearrange("(r p) -> p r", p=128))
            bv_sb = consts.tile([128, CL], F32)
            nc.gpsimd.dma_start(
                out=bv_sb,
                in_=bass.AP(tensor=bv, offset=0, ap=[[0, 128], [1, CL]]),
            )

            # ---- phase 1: QKV ---------------------------------------------
            qt_sb = qkvp.tile([128, NRT, T], MMD, tag="qt")   # Q^T
            kt_sb = qkvp.tile([128, NRT, T], MMD, tag="kt")   # K^T
            v_sb = qkvp.tile([128, NTT, HL, D + 1], MMD, tag="v")  # [V_h | 1]
            nc.vector.tensor_copy(
                v_sb[:, :, :, D:D + 1],
                one_f32.rearrange("p (t h e) -> p t h e", t=NTT, h=HL))

            _qkv_phase(nc, tc, xt, wq, wk, wv, qt_sb, kt_sb, v_sb,
                       bq_sb, bk_sb, bv_sb, ps_m_pool)

            wp_sb = qkvp.tile([128, NRT, C], MMD, tag="wp")
            nc.sync.dma_start(
                out=wp_sb, in_=wp[:].rearrange("(r p) n -> p r n", p=128))

            # ---- phase 2+3: attention + proj, chunk-major ------------------
            with (
                tc.tile_pool(name="pt", bufs=10) as pt_pool,
                tc.tile_pool(name="yt", bufs=2) as yt_pool,
                tc.tile_pool(name="work", bufs=2) as work,
                tc.tile_pool(name="osb", bufs=2) as osb_pool,
            ):
                for ch in range(NCH):
                    n_kt = 4 * (ch + 1)      # k-tiles 0..4ch+3 are <= chunk
                    q0 = ch * QCH
                    yt_c = yt_pool.tile([128, NRT, QCH], MMD, tag="yt")
                    for h in range(HL):
                        hp = (h % 2) * D     # partition offset of head rows
                        hr = h // 2          # row-tile of head rows
                        q_h = qt_sb[hp:hp + D, hr, q0:q0 + QCH]
                        # S^T + exp, two k-tiles at a time
                        pt_tiles = []
                        for half in range(n_kt // 2):
                            ps_s = ps_s_pool.tile([128, 2, 512], F32, tag="s")
                            for j in range(2):
                                kt = 2 * half + j
                                nc.tensor.matmul(
                                    ps_s[:, j, :],
                                    lhsT=kt_sb[hp:hp + D, hr,
                                                   kt * 128:(kt + 1) * 128],
                                    rhs=q_h,
                                    start=True, stop=True,
                                )
                            pt = pt_pool.tile([128, 2, 512], MMD, tag="pt")
                            nc.scalar.activation(
                                pt, ps_s, mybir.ActivationFunctionType.Exp,
                                scale=SCALE)
                            pt_tiles.append(pt)
                        # causal fixup on the 4 diagonal k-tiles (kt = 4ch+j)
                        for j in range(4):
                            kt = 4 * ch + j
                            blk = pt_tiles[kt // 2][:, kt % 2, :]
                            if j > 0:
                                nc.vector.tensor_copy(
                                    blk[:, 0:j * 128], zero_f32[:, 0:j * 128])
                            nc.vector.tensor_mul(
                                blk[:, j * 128:(j + 1) * 128],
                                blk[:, j * 128:(j + 1) * 128], diag)
                        # PV: y^T[0:64] = sum_k V P^T ; row 64 = denominator
                        ps_y = ps_y_pool.tile([128, 512], F32, tag="y")
                        for kt in range(n_kt):
                            nc.tensor.matmul(
                                ps_y[0:D + 1, :],
                                lhsT=v_sb[:, kt, h, :],
                                rhs=pt_tiles[kt // 2][:, kt % 2, :],
                                start=(kt == 0), stop=(kt == n_kt - 1),
                            )
                        # normalize: yt = y^T * (1/denom) bcast over 64 rows
                        rec = work.tile([128, 512], MMD, tag="rec")
                        with nc.allow_low_precision("f32r reciprocal, ~1e-4"):
                            nc.vector.reciprocal(rec[0:1, :],
                                                 ps_y[D:D + 1, :])
                        ps_b = ps_m_pool.tile([128, 512], F32, tag="mm")
                        nc.tensor.matmul(
                            ps_b[0:D, :], lhsT=ones[0:1, :],
                            rhs=rec[0:1, :], start=True, stop=True)
                        rb = work.tile([128, 512], F32, tag="rb")
                        nc.scalar.activation(
                            rb[0:D, :], ps_b[0:D, :],
                            mybir.ActivationFunctionType.Copy)
                        nc.vector.tensor_mul(
                            yt_c[hp:hp + D, hr, :], ps_y[0:D, :], rb[0:D, :])

                    # ---- projection for this token chunk ----
                    for ts in range(QCH // 128):
                        o_sb = osb_pool.tile([128, C], F32, tag="o")
                        for nb in range(2):
                            ps_o = ps_m_pool.tile([128, 512], F32, tag="mm")
                            for ct in range(NRT):
                                nc.tensor.matmul(
                                    ps_o,
                                    lhsT=yt_c[:, ct,
                                                  ts * 128:(ts + 1) * 128],
                                    rhs=wp_sb[:, ct,
                                                  nb * 512:(nb + 1) * 512],
                                    start=(ct == 0), stop=(ct == NRT - 1),
                                )
                            nc.vector.tensor_copy(
                                o_sb[:, nb * 512:(nb + 1) * 512], ps_o)
                        r0 = q0 + ts * 128
                        nc.sync.dma_start(out=out[r0:r0 + 128, :], in_=o_sb)
    nc.compile()
    return nc


_NC = None


def _get_nc():
    global _NC
    if _NC is None:
        _NC = build_nc()
    return _NC


def _make_in_maps(x, W_attn, b_attn, W_proj):
    x = np.ascontiguousarray(np.asarray(x, dtype=np.float32))
    W_attn = np.asarray(W_attn, dtype=np.float32)
    b_attn = np.asarray(b_attn, dtype=np.float32)
    W_proj = np.asarray(W_proj, dtype=np.float32)
    in_maps = []
    for core in range(8):
        b, g = core // 2, core % 2
        s = slice(g * CL, (g + 1) * CL)
        in_maps.append({
            "xt": np.ascontiguousarray(x[b].T),
            "wq": np.ascontiguousarray(W_attn[:, 0 * C:1 * C][:, s]),
            "wk": np.ascontiguousarray(W_attn[:, 1 * C:2 * C][:, s]),
            "wv": np.ascontiguousarray(W_attn[:, 2 * C:3 * C][:, s]),
            "wp": np.ascontiguousarray(W_proj[s, :]),
            "bq": np.ascontiguousarray(b_attn[0 * C:1 * C][s]),
            "bk": np.ascontiguousarray(b_attn[1 * C:2 * C][s]),
            "bv": np.ascontiguousarray(b_attn[2 * C:3 * C][s]),
        })
    return in_maps


def _gather(results, b_proj):
    b_proj = np.asarray(b_proj, dtype=np.float32)
    out = np.empty((4, T, C), dtype=np.float32)
    for b in range(4):
        out[b] = results[2 * b]["out"] + results[2 * b + 1]["out"] + b_proj
    return out


def run(x, W_attn, b_attn, W_proj, b_proj, trace=False):
    nc = _get_nc()
    in_maps = _make_in_maps(x, W_attn, b_attn, W_proj)
    res = run_bass_kernel_spmd(nc, in_maps, list(range(8)), trace=trace)
    return _gather(res.results, b_proj), res


def kernel(x, W_attn, b_attn, W_proj, b_proj):
    out, _ = run(x, W_attn, b_attn, W_proj, b_proj)
    return out


# revision 47
# speedup vs baseline: 336.8849x; 336.8849x over previous
"""Causal self-attention (B=4, T=2048, C=1024, H=16) on 8 trn2 NeuronCores.

Sharding: core i = 2*b + g handles batch b (of 4) and head-group g (of 2,
8 heads each).  Inside each core:
  phase 1: QKV projection.  x^T (pre-transposed on host, [C, T]) resident in
           SBUF; Q^T/K^T computed head-dim-major [512, T]; V token-major
           [T, 512] with a ones column appended per head ([V_h | 1]).
  phase 2: attention per (head, q-chunk of 512).  Scores computed transposed
           (S^T[k, q] = K Q^T) so the softmax axis (k) is the partition dim
           of the PV matmul; exp on ScalarE (no max subtraction needed at
           these magnitudes); causal handled by a triangular mask on diagonal
           128x128 blocks + zeroing below-diagonal blocks; PV produces
           y^T[d, q] with row 64 = softmax denominator (from the ones col);
           normalization = reciprocal + ones-matmul broadcast + multiply.
  phase 3: projection y^T @ W_proj rows -> per-core partial [T, C].
Host sums the two partials per batch and adds b_proj.
"""

import os
import sys

for _p in ("/opt/trn_rl_repo", "/opt/pypackages"):
    if _p not in sys.path and os.path.isdir(_p):
        sys.path.append(_p)

import numpy as np

import concourse.bass as bass
import concourse.bacc as bacc
import concourse.mybir as mybir
from concourse.tile import TileContext
from concourse.bass_utils import run_bass_kernel_spmd

F32 = mybir.dt.float32
# Matmul operand dtype: float32r streams fp32 at 1 cycle/row (vs 4 for plain
# fp32) when the moving free dim is >= 256, at ~tf32 precision (~1.5e-4 rel
# err measured on HW for K=128).  Every producer of an fp32r matmul operand
# must itself write float32r (BIR verifier rule), so operand tiles and their
# DRAM sources are declared float32r.  Flip to F32 if accuracy fails.
MMD = mybir.dt.float32r

T = 2048          # tokens
C = 1024          # embed dim
D = 64            # head dim
HL = 8            # heads per core
CL = HL * D       # 512 local channels
FT = C // 128     # 8 feature tiles
NRT = CL // 128   # 4 row tiles of Q^T/K^T/y^T
NTT = T // 128    # 16 token tiles
QCH = 512         # q chunk
NCH = T // QCH    # 4 chunks
SCALE = 1.0 / 8.0  # 1/sqrt(D)


def _qkv_phase(nc, tc, xt, wq, wk, wv, bq, bk, bv,
               qt_sb, kt_sb, v_sb, bq_sb, bk_sb, bv_sb):
    with (
        tc.tile_pool(name="xtw", bufs=2) as xtw,
        tc.tile_pool(name="ps_q", bufs=8, space="PSUM") as ps_q_pool,
    ):
        wq_sb = xtw.tile([128, FT, CL], MMD, tag="w")
        wq_r = wq[:].rearrange("(f p) n -> p f n", p=128)
        for rt in range(NRT):
            rs = slice(rt * 128, (rt + 1) * 128)
            nc.scalar.dma_start(out=wq_sb[:, :, rs], in_=wq_r[:, :, rs])
        xt_sb = xtw.tile([128, FT, T], MMD, tag="xt", bufs=1)
        xt_r = xt[:].rearrange("(f p) n -> p f n", p=128)
        for ft in range(FT):
            for hv in range(2):
                hs = slice(hv * (T // 2), (hv + 1) * (T // 2))
                nc.sync.dma_start(out=xt_sb[:, ft, hs], in_=xt_r[:, ft, hs])

        for w_sb, dest, bias in ((wq_sb, qt_sb, bq_sb), (None, kt_sb, bk_sb)):
            if w_sb is None:
                w_sb = xtw.tile([128, FT, CL], MMD, tag="w", name="wk_sb")
                with tc.tile_wait_until(0.025):
                    nc.scalar.dma_start(
                        out=w_sb,
                        in_=wk[:].rearrange("(f p) n -> p f n", p=128))
            for rt in range(NRT):
                for nt in range(NCH):
                    ps = ps_q_pool.tile([128, 512], F32, tag="q")
                    for ft in range(FT):
                        nc.tensor.matmul(
                            ps,
                            lhsT=w_sb[:, ft, rt * 128:(rt + 1) * 128],
                            rhs=xt_sb[:, ft, nt * 512:(nt + 1) * 512],
                            start=(ft == 0), stop=(ft == FT - 1),
                        )
                    nc.vector.tensor_scalar_add(
                        dest[:, rt, nt * 512:(nt + 1) * 512],
                        ps, bias[:, rt:rt + 1])

        wv_sb = xtw.tile([128, FT, CL], MMD, tag="w")
        nc.scalar.dma_start(
            out=wv_sb, in_=wv[:].rearrange("(f p) n -> p f n", p=128))
        bv_r = bv_sb.rearrange("p (h d) -> p h d", d=D)
        for tt in range(NTT):
            ps = ps_q_pool.tile([128, 512], F32, tag="q")
            for ft in range(FT):
                nc.tensor.matmul(
                    ps,
                    lhsT=xt_sb[:, ft, tt * 128:(tt + 1) * 128],
                    rhs=wv_sb[:, ft, :],
                    start=(ft == 0), stop=(ft == FT - 1),
                )
            nc.vector.tensor_add(
                v_sb[:, tt, :, 0:D],
                ps.rearrange("p (h d) -> p h d", d=D), bv_r)


def build_nc():
    nc = bacc.Bacc()
    xt = nc.declare_dram_parameter("xt", [C, T], MMD, isOutput=False)
    wq = nc.declare_dram_parameter("wq", [C, CL], MMD, isOutput=False)
    wk = nc.declare_dram_parameter("wk", [C, CL], MMD, isOutput=False)
    wv = nc.declare_dram_parameter("wv", [C, CL], MMD, isOutput=False)
    wp = nc.declare_dram_parameter("wp", [CL, C], MMD, isOutput=False)
    bq = nc.declare_dram_parameter("bq", [CL], F32, isOutput=False)
    bk = nc.declare_dram_parameter("bk", [CL], F32, isOutput=False)
    bv = nc.declare_dram_parameter("bv", [CL], F32, isOutput=False)
    out = nc.declare_dram_parameter("out", [T, C], F32, isOutput=True)

    with TileContext(nc) as tc:
        with (
            tc.tile_pool(name="consts", bufs=1) as consts,
            tc.tile_pool(name="qkv", bufs=1) as qkvp,
        ):
            # ---- constants -------------------------------------------------
            # diag_mask[p, c] = 1.0 if p <= c else 0.0  (valid = k <= q)
            diag = consts.tile([128, 128], F32)
            nc.gpsimd.memset(diag, 1.0)
            nc.gpsimd.affine_select(
                out=diag, in_=diag,
                compare_op=mybir.AluOpType.is_ge,
                fill=0.0, base=0,
                pattern=[[1, 128]], channel_multiplier=-1,
            )
            one_f32 = consts.tile([128, 128], F32, tag="one_f32")
            nc.vector.memset(one_f32, 1.0)
            zero_f32 = consts.tile([128, 384], F32, tag="zero_f32")
            nc.vector.memset(zero_f32, 0.0)
            ones = consts.tile([128, D], MMD)
            nc.vector.tensor_copy(ones, one_f32[:, 0:D])

            bq_sb = consts.tile([128, NRT], F32)
            bk_sb = consts.tile([128, NRT], F32)
            nc.sync.dma_start(out=bq_sb,
                              in_=bq[:].rearrange("(r p) -> p r", p=128))
            nc.sync.dma_start(out=bk_sb,
                              in_=bk[:].rearrange("(r p) -> p r", p=128))
            bv_sb = consts.tile([128, CL], F32)
            nc.gpsimd.dma_start(
                out=bv_sb,
                in_=bass.AP(tensor=bv, offset=0, ap=[[0, 128], [1, CL]]),
            )

            # ---- phase 1: QKV ---------------------------------------------
            qt_sb = qkvp.tile([128, NRT, T], MMD, tag="qt")   # Q^T
            kt_sb = qkvp.tile([128, NRT, T], MMD, tag="kt")   # K^T
            v_sb = qkvp.tile([128, NTT, HL, D + 1], MMD, tag="v")  # [V_h | 1]
            nc.vector.tensor_copy(
                v_sb[:, :, :, D:D + 1],
                one_f32.rearrange("p (t h e) -> p t h e", t=NTT, h=HL))

            _qkv_phase(nc, tc, xt, wq, wk, wv, bq, bk, bv,
                       qt_sb, kt_sb, v_sb, bq_sb, bk_sb, bv_sb)

            # ---- phase 2+3: attention + proj, chunk-major ------------------
            with (
                tc.tile_pool(name="ps_s", bufs=2, space="PSUM") as ps_s_pool,
                tc.tile_pool(name="ps_y", bufs=2, space="PSUM") as ps_y_pool,
                tc.tile_pool(name="ps_m", bufs=2, space="PSUM") as ps_m_pool,
                tc.tile_pool(name="wpp", bufs=1) as wpp,
                tc.tile_pool(name="pt", bufs=9) as pt_pool,
                tc.tile_pool(name="yt", bufs=2) as yt_pool,
                tc.tile_pool(name="work", bufs=3) as work,
                tc.tile_pool(name="osb", bufs=3) as osb_pool,
            ):
                wp_sb = wpp.tile([128, NRT, C], MMD, tag="wp")
                nc.sync.dma_start(
                    out=wp_sb, in_=wp[:].rearrange("(r p) n -> p r n", p=128))

                def proj_group(yt_p, p_q0, ts):
                    o_sb = osb_pool.tile([128, C], F32, tag="o", name="o_sb")
                    for nb in range(2):
                        ps_o = ps_m_pool.tile([128, 512], F32, tag="mm",
                                              name="ps_o")
                        for ct in range(NRT):
                            nc.tensor.matmul(
                                ps_o,
                                lhsT=yt_p[:, ct, ts * 128:(ts + 1) * 128],
                                rhs=wp_sb[:, ct, nb * 512:(nb + 1) * 512],
                                start=(ct == 0), stop=(ct == NRT - 1),
                            )
                        nc.vector.tensor_copy(
                            o_sb[:, nb * 512:(nb + 1) * 512], ps_o)
                    r0 = p_q0 + ts * 128
                    nc.sync.dma_start(out=out[r0:r0 + 128, :], in_=o_sb)

                prev_yt = None
                prev_q0 = 0
                for ch in range(NCH):
                    n_kt = 4 * (ch + 1)      # k-tiles 0..4ch+3 are <= chunk
                    q0 = ch * QCH
                    yt_c = yt_pool.tile([128, NRT, QCH], MMD, tag="yt")
                    # head pair (2j, 2j+1) = partitions 0:64 / 64:128 of
                    # row-tile j.  The two S matmuls per k-tile use disjoint
                    # PE row groups (base partition 0 vs 64), so they run
                    # concurrently on the array (K=64 row packing).
                    for j in range(NRT):
                        ps_ys = []
                        for hh in range(2):
                            ps_y = ps_y_pool.tile([128, 512], F32, tag="y")
                            ps_ys.append(ps_y)
                        pt_tiles = {}

                        def pv(kt):
                            dj = kt - 4 * ch
                            qs = dj * 128 if dj > 0 else 0
                            for hh in range(2):
                                nc.tensor.matmul(
                                    ps_ys[hh][0:D + 1, qs:],
                                    lhsT=v_sb[:, kt, 2 * j + hh, :],
                                    rhs=pt_tiles.pop(kt)[:, hh, qs:]
                                    if hh else pt_tiles[kt][:, hh, qs:],
                                    start=(kt == 0), stop=(kt == n_kt - 1),
                                )

                        # PV trails S by LAG k-tiles so PE never waits on
                        # exp; shorter for small chunks so PVs still overlap.
                        LAG = min(4, n_kt // 2)
                        for kt in range(n_kt):
                            kc = slice(kt * 128, (kt + 1) * 128)
                            dj = kt - 4 * ch  # diagonal block index, if >= 0
                            # valid q-span of this k-tile within the chunk:
                            # below-diagonal columns are fully masked, skip
                            # computing and exponentiating them.
                            qs = dj * 128 if dj > 0 else 0
                            ps_s = ps_s_pool.tile([128, 2, 512], F32, tag="s")
                            for hh in range(2):
                                hp = hh * D
                                nc.tensor.matmul(
                                    ps_s[:, hh, qs:],
                                    lhsT=kt_sb[hp:hp + D, j, kc],
                                    rhs=qt_sb[hp:hp + D, j,
                                              q0 + qs:q0 + QCH],
                                    start=True, stop=True,
                                )
                            pt = pt_pool.tile([128, 2, 512], MMD, tag="pt")
                            nc.scalar.activation(
                                pt[:, :, qs:], ps_s[:, :, qs:],
                                mybir.ActivationFunctionType.Exp,
                                scale=SCALE)
                            pt_tiles[kt] = pt
                            if dj >= 0:
                                # cols < qs are never read by pv(); only the
                                # diagonal 128-block needs the triangular mask
                                for hh in range(2):
                                    blk = pt[:, hh, :]
                                    nc.vector.tensor_mul(
                                        blk[:, dj * 128:(dj + 1) * 128],
                                        blk[:, dj * 128:(dj + 1) * 128], diag)
                            if kt >= LAG:
                                pv(kt - LAG)
                            if kt == 1 and prev_yt is not None:
                                # previous chunk's projection fills the PE
                                # while the first exps stream on ACT
                                proj_group(prev_yt, prev_q0, j)
                        for kt in range(max(0, n_kt - LAG), n_kt):
                            pv(kt)
                        # normalize: yt = y^T * (1/denom) bcast over 64 rows
                        for hh in range(2):
                            ps_y = ps_ys[hh]
                            hp = hh * D
                            # one copy frees the PSUM slot for the next pair
                            yc = work.tile([128, 512], F32, tag="yc")
                            nc.vector.tensor_copy(yc[0:D + 1, :],
                                                  ps_y[0:D + 1, :])
                            rec = work.tile([128, 512], MMD, tag="rec")
                            with nc.allow_low_precision("f32r recip ~1e-4"):
                                nc.vector.reciprocal(rec[0:1, :],
                                                     yc[D:D + 1, :])
                            ps_b = ps_m_pool.tile([128, 512], F32, tag="mm")
                            nc.tensor.matmul(
                                ps_b[0:D, :], lhsT=ones[0:1, :],
                                rhs=rec[0:1, :], start=True, stop=True)
                            rb = work.tile([128, 512], F32, tag="rb")
                            nc.vector.tensor_copy(rb[0:D, :], ps_b[0:D, :])
                            nc.vector.tensor_mul(
                                yt_c[hp:hp + D, j, :], yc[0:D, :],
                                rb[0:D, :])
                    prev_yt, prev_q0 = yt_c, q0
                # tail: last chunk's projection
                for ts in range(QCH // 128):
                    proj_group(prev_yt, prev_q0, ts)
    nc.compile()
    return nc


_NC = None


def _get_nc():
    global _NC
    if _NC is None:
        _NC = build_nc()
    return _NC


def _make_in_maps(x, W_attn, b_attn, W_proj):
    x = np.ascontiguousarray(np.asarray(x, dtype=np.float32))
    W_attn = np.asarray(W_attn, dtype=np.float32)
    b_attn = np.asarray(b_attn, dtype=np.float32)
    W_proj = np.asarray(W_proj, dtype=np.float32)
    in_maps = []
    for core in range(8):
        b, g = core // 2, core % 2
        s = slice(g * CL, (g + 1) * CL)
        in_maps.append({
            "xt": np.ascontiguousarray(x[b].T),
            "wq": np.ascontiguousarray(W_attn[:, 0 * C:1 * C][:, s]),
            "wk": np.ascontiguousarray(W_attn[:, 1 * C:2 * C][:, s]),
            "wv": np.ascontiguousarray(W_attn[:, 2 * C:3 * C][:, s]),
            "wp": np.ascontiguousarray(W_proj[s, :]),
            "bq": np.ascontiguousarray(b_attn[0 * C:1 * C][s]),
            "bk": np.ascontiguousarray(b_attn[1 * C:2 * C][s]),
            "bv": np.ascontiguousarray(b_attn[2 * C:3 * C][s]),
        })
    return in_maps


def _gather(results, b_proj):
    b_proj = np.asarray(b_proj, dtype=np.float32)
    out = np.empty((4, T, C), dtype=np.float32)
    for b in range(4):
        out[b] = results[2 * b]["out"] + results[2 * b + 1]["out"] + b_proj
    return out


def run(x, W_attn, b_attn, W_proj, b_proj, trace=False):
    """Reference path via run_bass_kernel_spmd (re-traces every call)."""
    nc = _get_nc()
    in_maps = _make_in_maps(x, W_attn, b_attn, W_proj)
    res = run_bass_kernel_spmd(nc, in_maps, list(range(8)), trace=trace)
    return _gather(res.results, b_proj), res


class _Runner:
    """Cached PJRT executor: builds the sharded jit once, reuses it.

    No output donation: the kernel writes every element of "out", so the
    pre-zeroed output operand run_bass_kernel_spmd donates is unnecessary.
    """

    def __init__(self, nc, n_cores=8):
        import jax
        from jax.experimental.shard_map import shard_map
        from jax.sharding import Mesh, NamedSharding, PartitionSpec
        from concourse.bass2jax import (
            _bass_exec_p, install_neuronx_cc_hook, partition_id_tensor)

        install_neuronx_cc_hook()
        self.jax = jax
        self.nc = nc
        self.n_cores = n_cores
        in_names, out_names, out_avals = [], [], []
        for alloc in nc.m.functions[0].allocations:
            if not isinstance(alloc, mybir.MemoryLocationSet):
                continue
            name = alloc.memorylocations[0].name
            if alloc.kind == "ExternalInput":
                if name != "partition_id":
                    in_names.append(name)
            elif alloc.kind == "ExternalOutput":
                out_names.append(name)
                out_avals.append(jax.core.ShapedArray(
                    tuple(alloc.tensor_shape), mybir.dt.np(alloc.dtype)))
        self.in_names = in_names
        self.out_names = out_names
        self.out_avals = out_avals
        all_in = in_names + out_names + ["partition_id"]
        n_ops = len(in_names) + len(out_names)

        def _body(*args):
            outs = _bass_exec_p.bind(
                *args, partition_id_tensor(),
                out_avals=tuple(out_avals),
                in_names=tuple(all_in),
                out_names=tuple(out_names),
                lowering_input_output_aliases=(),
                sim_require_finite=True,
                sim_require_nnan=True,
                nc=nc,
            )
            return tuple(outs)

        devices = jax.devices()[:n_cores]
        self.mesh = Mesh(np.asarray(devices), ("core",))
        spec = PartitionSpec("core")
        self.sharding = NamedSharding(self.mesh, spec)
        self.fn = jax.jit(
            shard_map(_body, mesh=self.mesh, in_specs=(spec,) * n_ops,
                      out_specs=(spec,) * len(out_names), check_rep=False),
            keep_unused=True)
        # device-resident zeros, reused every call (read-only operand)
        self.zero_out = [
            jax.device_put(
                np.zeros((n_cores * av.shape[0], *av.shape[1:]), av.dtype),
                self.sharding)
            for av in out_avals
        ]

    def __call__(self, in_maps):
        n = self.n_cores
        concat_in = [
            np.concatenate([np.asarray(in_maps[c][name]) for c in range(n)],
                           axis=0)
            for name in self.in_names
        ]
        outs = self.fn(*concat_in, *self.zero_out)
        out = np.asarray(outs[0]).reshape(n, *self.out_avals[0].shape)
        return [{self.out_names[0]: out[c]} for c in range(n)]


_RUNNER = None


def _get_runner():
    global _RUNNER
    if _RUNNER is None:
        _RUNNER = _Runner(_get_nc())
    return _RUNNER


def kernel(x, W_attn, b_attn, W_proj, b_proj):
    in_maps = _make_in_maps(x, W_attn, b_attn, W_proj)
    try:
        results = _get_runner()(in_maps)
    except Exception:
        res = run_bass_kernel_spmd(_get_nc(), in_maps, list(range(8)))
        results = res.results
    return _gather(results, b_proj)


# revision 48
# speedup vs baseline: 337.7171x; 1.0025x over previous
"""Causal self-attention (B=4, T=2048, C=1024, H=16) on 8 trn2 NeuronCores.

Sharding: core i = 2*b + g handles batch b (of 4) and head-group g (of 2,
8 heads each).  Inside each core:
  phase 1: QKV projection.  x^T (pre-transposed on host, [C, T]) resident in
           SBUF; Q^T/K^T computed head-dim-major [512, T]; V token-major
           [T, 512] with a ones column appended per head ([V_h | 1]).
  phase 2: attention per (head, q-chunk of 512).  Scores computed transposed
           (S^T[k, q] = K Q^T) so the softmax axis (k) is the partition dim
           of the PV matmul; exp on ScalarE (no max subtraction needed at
           these magnitudes); causal handled by a triangular mask on diagonal
           128x128 blocks + zeroing below-diagonal blocks; PV produces
           y^T[d, q] with row 64 = softmax denominator (from the ones col);
           normalization = reciprocal + ones-matmul broadcast + multiply.
  phase 3: projection y^T @ W_proj rows -> per-core partial [T, C].
Host sums the two partials per batch and adds b_proj.
"""

import os
import sys

for _p in ("/opt/trn_rl_repo", "/opt/pypackages"):
    if _p not in sys.path and os.path.isdir(_p):
        sys.path.append(_p)

import numpy as np

import concourse.bass as bass
import concourse.bacc as bacc
import concourse.mybir as mybir
from concourse.tile import TileContext
from concourse.bass_utils import run_bass_kernel_spmd

F32 = mybir.dt.float32
# Matmul operand dtype: float32r streams fp32 at 1 cycle/row (vs 4 for plain
# fp32) when the moving free dim is >= 256, at ~tf32 precision (~1.5e-4 rel
# err measured on HW for K=128).  Every producer of an fp32r matmul operand
# must itself write float32r (BIR verifier rule), so operand tiles and their
# DRAM sources are declared float32r.  Flip to F32 if accuracy fails.
MMD = mybir.dt.float32r

T = 2048          # tokens
C = 1024          # embed dim
D = 64            # head dim
HL = 8            # heads per core
CL = HL * D       # 512 local channels
FT = C // 128     # 8 feature tiles
NRT = CL // 128   # 4 row tiles of Q^T/K^T/y^T
NTT = T // 128    # 16 token tiles
QCH = 512         # q chunk
NCH = T // QCH    # 4 chunks
SCALE = 1.0 / 8.0  # 1/sqrt(D)


def _qkv_phase(nc, tc, xt, wq, wk, wv, bq, bk, bv,
               qt_sb, kt_sb, v_sb, bq_sb, bk_sb, bv_sb):
    with (
        tc.tile_pool(name="xtw", bufs=2) as xtw,
        tc.tile_pool(name="ps_q", bufs=8, space="PSUM") as ps_q_pool,
    ):
        wq_sb = xtw.tile([128, FT, CL], MMD, tag="w")
        wq_r = wq[:].rearrange("(f p) n -> p f n", p=128)
        for rt in range(NRT):
            rs = slice(rt * 128, (rt + 1) * 128)
            nc.scalar.dma_start(out=wq_sb[:, :, rs], in_=wq_r[:, :, rs])
        xt_sb = xtw.tile([128, FT, T], MMD, tag="xt", bufs=1)
        xt_r = xt[:].rearrange("(f p) n -> p f n", p=128)
        for ft in range(FT):
            for hv in range(2):
                hs = slice(hv * (T // 2), (hv + 1) * (T // 2))
                nc.sync.dma_start(out=xt_sb[:, ft, hs], in_=xt_r[:, ft, hs])

        for w_sb, dest, bias in ((wq_sb, qt_sb, bq_sb), (None, kt_sb, bk_sb)):
            if w_sb is None:
                w_sb = xtw.tile([128, FT, CL], MMD, tag="w", name="wk_sb")
                with tc.tile_wait_until(0.025):
                    nc.scalar.dma_start(
                        out=w_sb,
                        in_=wk[:].rearrange("(f p) n -> p f n", p=128))
            for rt in range(NRT):
                for nt in range(NCH):
                    ps = ps_q_pool.tile([128, 512], F32, tag="q")
                    for ft in range(FT):
                        nc.tensor.matmul(
                            ps,
                            lhsT=w_sb[:, ft, rt * 128:(rt + 1) * 128],
                            rhs=xt_sb[:, ft, nt * 512:(nt + 1) * 512],
                            start=(ft == 0), stop=(ft == FT - 1),
                        )
                    nc.vector.tensor_scalar_add(
                        dest[:, rt, nt * 512:(nt + 1) * 512],
                        ps, bias[:, rt:rt + 1])

        wv_sb = xtw.tile([128, FT, CL], MMD, tag="w")
        nc.scalar.dma_start(
            out=wv_sb, in_=wv[:].rearrange("(f p) n -> p f n", p=128))
        bv_r = bv_sb.rearrange("p (h d) -> p h d", d=D)
        for tt in range(NTT):
            ps = ps_q_pool.tile([128, 512], F32, tag="q")
            for ft in range(FT):
                nc.tensor.matmul(
                    ps,
                    lhsT=xt_sb[:, ft, tt * 128:(tt + 1) * 128],
                    rhs=wv_sb[:, ft, :],
                    start=(ft == 0), stop=(ft == FT - 1),
                )
            nc.vector.tensor_add(
                v_sb[:, tt, :, 0:D],
                ps.rearrange("p (h d) -> p h d", d=D), bv_r)


def build_nc():
    nc = bacc.Bacc()
    xt = nc.declare_dram_parameter("xt", [C, T], MMD, isOutput=False)
    wq = nc.declare_dram_parameter("wq", [C, CL], MMD, isOutput=False)
    wk = nc.declare_dram_parameter("wk", [C, CL], MMD, isOutput=False)
    wv = nc.declare_dram_parameter("wv", [C, CL], MMD, isOutput=False)
    wp = nc.declare_dram_parameter("wp", [CL, C], MMD, isOutput=False)
    bq = nc.declare_dram_parameter("bq", [CL], F32, isOutput=False)
    bk = nc.declare_dram_parameter("bk", [CL], F32, isOutput=False)
    bv = nc.declare_dram_parameter("bv", [CL], F32, isOutput=False)
    out = nc.declare_dram_parameter("out", [T, C], F32, isOutput=True)

    with TileContext(nc) as tc:
        with (
            tc.tile_pool(name="consts", bufs=1) as consts,
            tc.tile_pool(name="qkv", bufs=1) as qkvp,
        ):
            # ---- constants -------------------------------------------------
            # diag_mask[p, c] = 1.0 if p <= c else 0.0  (valid = k <= q)
            diag = consts.tile([128, 128], F32)
            nc.gpsimd.memset(diag, 1.0)
            nc.gpsimd.affine_select(
                out=diag, in_=diag,
                compare_op=mybir.AluOpType.is_ge,
                fill=0.0, base=0,
                pattern=[[1, 128]], channel_multiplier=-1,
            )
            one_f32 = consts.tile([128, 128], F32, tag="one_f32")
            nc.vector.memset(one_f32, 1.0)
            zero_f32 = consts.tile([128, 384], F32, tag="zero_f32")
            nc.vector.memset(zero_f32, 0.0)
            ones = consts.tile([128, D], MMD)
            nc.vector.tensor_copy(ones, one_f32[:, 0:D])

            bq_sb = consts.tile([128, NRT], F32)
            bk_sb = consts.tile([128, NRT], F32)
            nc.sync.dma_start(out=bq_sb,
                              in_=bq[:].rearrange("(r p) -> p r", p=128))
            nc.sync.dma_start(out=bk_sb,
                              in_=bk[:].rearrange("(r p) -> p r", p=128))
            bv_sb = consts.tile([128, CL], F32)
            nc.gpsimd.dma_start(
                out=bv_sb,
                in_=bass.AP(tensor=bv, offset=0, ap=[[0, 128], [1, CL]]),
            )

            # ---- phase 1: QKV ---------------------------------------------
            qt_sb = qkvp.tile([128, NRT, T], MMD, tag="qt")   # Q^T
            kt_sb = qkvp.tile([128, NRT, T], MMD, tag="kt")   # K^T
            v_sb = qkvp.tile([128, NTT, HL, D + 1], MMD, tag="v")  # [V_h | 1]
            nc.vector.tensor_copy(
                v_sb[:, :, :, D:D + 1],
                one_f32.rearrange("p (t h e) -> p t h e", t=NTT, h=HL))

            _qkv_phase(nc, tc, xt, wq, wk, wv, bq, bk, bv,
                       qt_sb, kt_sb, v_sb, bq_sb, bk_sb, bv_sb)

            # ---- phase 2+3: attention + proj, chunk-major ------------------
            with (
                tc.tile_pool(name="ps_s", bufs=2, space="PSUM") as ps_s_pool,
                tc.tile_pool(name="ps_y", bufs=2, space="PSUM") as ps_y_pool,
                tc.tile_pool(name="ps_m", bufs=2, space="PSUM") as ps_m_pool,
                tc.tile_pool(name="wpp", bufs=1) as wpp,
                tc.tile_pool(name="pt", bufs=9) as pt_pool,
                tc.tile_pool(name="yt", bufs=2) as yt_pool,
                tc.tile_pool(name="work", bufs=3) as work,
                tc.tile_pool(name="osb", bufs=3) as osb_pool,
            ):
                wp_sb = wpp.tile([128, NRT, C], MMD, tag="wp")
                nc.sync.dma_start(
                    out=wp_sb, in_=wp[:].rearrange("(r p) n -> p r n", p=128))

                def proj_group(yt_p, p_q0, ts):
                    o_sb = osb_pool.tile([128, C], F32, tag="o", name="o_sb")
                    r0 = p_q0 + ts * 128
                    for nb in range(2):
                        ns = slice(nb * 512, (nb + 1) * 512)
                        ps_o = ps_m_pool.tile([128, 512], F32, tag="mm",
                                              name="ps_o")
                        for ct in range(NRT):
                            nc.tensor.matmul(
                                ps_o,
                                lhsT=yt_p[:, ct, ts * 128:(ts + 1) * 128],
                                rhs=wp_sb[:, ct, ns],
                                start=(ct == 0), stop=(ct == NRT - 1),
                            )
                        nc.vector.tensor_copy(o_sb[:, ns], ps_o)
                        nc.sync.dma_start(out=out[r0:r0 + 128, ns],
                                          in_=o_sb[:, ns])

                prev_yt = None
                prev_q0 = 0
                for ch in range(NCH):
                    n_kt = 4 * (ch + 1)      # k-tiles 0..4ch+3 are <= chunk
                    q0 = ch * QCH
                    yt_c = yt_pool.tile([128, NRT, QCH], MMD, tag="yt")
                    # head pair (2j, 2j+1) = partitions 0:64 / 64:128 of
                    # row-tile j.  The two S matmuls per k-tile use disjoint
                    # PE row groups (base partition 0 vs 64), so they run
                    # concurrently on the array (K=64 row packing).
                    for j in range(NRT):
                        ps_ys = []
                        for hh in range(2):
                            ps_y = ps_y_pool.tile([128, 512], F32, tag="y")
                            ps_ys.append(ps_y)
                        pt_tiles = {}

                        def pv(kt):
                            dj = kt - 4 * ch
                            qs = dj * 128 if dj > 0 else 0
                            for hh in range(2):
                                nc.tensor.matmul(
                                    ps_ys[hh][0:D + 1, qs:],
                                    lhsT=v_sb[:, kt, 2 * j + hh, :],
                                    rhs=pt_tiles.pop(kt)[:, hh, qs:]
                                    if hh else pt_tiles[kt][:, hh, qs:],
                                    start=(kt == 0), stop=(kt == n_kt - 1),
                                )

                        # PV trails S by LAG k-tiles so PE never waits on
                        # exp; shorter for small chunks so PVs still overlap.
                        LAG = min(4, n_kt // 2)
                        for kt in range(n_kt):
                            kc = slice(kt * 128, (kt + 1) * 128)
                            dj = kt - 4 * ch  # diagonal block index, if >= 0
                            # valid q-span of this k-tile within the chunk:
                            # below-diagonal columns are fully masked, skip
                            # computing and exponentiating them.
                            qs = dj * 128 if dj > 0 else 0
                            ps_s = ps_s_pool.tile([128, 2, 512], F32, tag="s")
                            for hh in range(2):
                                hp = hh * D
                                nc.tensor.matmul(
                                    ps_s[:, hh, qs:],
                                    lhsT=kt_sb[hp:hp + D, j, kc],
                                    rhs=qt_sb[hp:hp + D, j,
                                              q0 + qs:q0 + QCH],
                                    start=True, stop=True,
                                )
                            pt = pt_pool.tile([128, 2, 512], MMD, tag="pt")
                            nc.scalar.activation(
                                pt[:, :, qs:], ps_s[:, :, qs:],
                                mybir.ActivationFunctionType.Exp,
                                scale=SCALE)
                            pt_tiles[kt] = pt
                            if dj >= 0:
                                # cols < qs are never read by pv(); only the
                                # diagonal 128-block needs the triangular mask
                                for hh in range(2):
                                    blk = pt[:, hh, :]
                                    nc.vector.tensor_mul(
                                        blk[:, dj * 128:(dj + 1) * 128],
                                        blk[:, dj * 128:(dj + 1) * 128], diag)
                            if kt >= LAG:
                                pv(kt - LAG)
                            if kt == 1 and prev_yt is not None:
                                # previous chunk's projection fills the PE
                                # while the first exps stream on ACT
                                proj_group(prev_yt, prev_q0, j)
                        for kt in range(max(0, n_kt - LAG), n_kt):
                            pv(kt)
                        # normalize: yt = y^T * (1/denom) bcast over 64 rows
                        for hh in range(2):
                            ps_y = ps_ys[hh]
                            hp = hh * D
                            # one copy frees the PSUM slot for the next pair
                            yc = work.tile([128, 512], F32, tag="yc")
                            nc.vector.tensor_copy(yc[0:D + 1, :],
                                                  ps_y[0:D + 1, :])
                            rec = work.tile([128, 512], MMD, tag="rec")
                            with nc.allow_low_precision("f32r recip ~1e-4"):
                                nc.vector.reciprocal(rec[0:1, :],
                                                     yc[D:D + 1, :])
                            ps_b = ps_m_pool.tile([128, 512], F32, tag="mm")
                            nc.tensor.matmul(
                                ps_b[0:D, :], lhsT=ones[0:1, :],
                                rhs=rec[0:1, :], start=True, stop=True)
                            rb = work.tile([128, 512], F32, tag="rb")
                            nc.vector.tensor_copy(rb[0:D, :], ps_b[0:D, :])
                            nc.vector.tensor_mul(
                                yt_c[hp:hp + D, j, :], yc[0:D, :],
                                rb[0:D, :])
                    prev_yt, prev_q0 = yt_c, q0
                # tail: last chunk's projection
                for ts in range(QCH // 128):
                    proj_group(prev_yt, prev_q0, ts)
    nc.compile()
    return nc


_NC = None


def _get_nc():
    global _NC
    if _NC is None:
        _NC = build_nc()
    return _NC


def _make_in_maps(x, W_attn, b_attn, W_proj):
    x = np.ascontiguousarray(np.asarray(x, dtype=np.float32))
    W_attn = np.asarray(W_attn, dtype=np.float32)
    b_attn = np.asarray(b_attn, dtype=np.float32)
    W_proj = np.asarray(W_proj, dtype=np.float32)
    in_maps = []
    for core in range(8):
        b, g = core // 2, core % 2
        s = slice(g * CL, (g + 1) * CL)
        in_maps.append({
            "xt": np.ascontiguousarray(x[b].T),
            "wq": np.ascontiguousarray(W_attn[:, 0 * C:1 * C][:, s]),
            "wk": np.ascontiguousarray(W_attn[:, 1 * C:2 * C][:, s]),
            "wv": np.ascontiguousarray(W_attn[:, 2 * C:3 * C][:, s]),
            "wp": np.ascontiguousarray(W_proj[s, :]),
            "bq": np.ascontiguousarray(b_attn[0 * C:1 * C][s]),
            "bk": np.ascontiguousarray(b_attn[1 * C:2 * C][s]),
            "bv": np.ascontiguousarray(b_attn[2 * C:3 * C][s]),
        })
    return in_maps


def _gather(results, b_proj):
    b_proj = np.asarray(b_proj, dtype=np.float32)
    out = np.empty((4, T, C), dtype=np.float32)
    for b in range(4):
        out[b] = results[2 * b]["out"] + results[2 * b + 1]["out"] + b_proj
    return out


def run(x, W_attn, b_attn, W_proj, b_proj, trace=False):
    """Reference path via run_bass_kernel_spmd (re-traces every call)."""
    nc = _get_nc()
    in_maps = _make_in_maps(x, W_attn, b_attn, W_proj)
    res = run_bass_kernel_spmd(nc, in_maps, list(range(8)), trace=trace)
    return _gather(res.results, b_proj), res


class _Runner:
    """Cached PJRT executor: builds the sharded jit once, reuses it.

    No output donation: the kernel writes every element of "out", so the
    pre-zeroed output operand run_bass_kernel_spmd donates is unnecessary.
    """

    def __init__(self, nc, n_cores=8):
        import jax
        from jax.experimental.shard_map import shard_map
        from jax.sharding import Mesh, NamedSharding, PartitionSpec
        from concourse.bass2jax import (
            _bass_exec_p, install_neuronx_cc_hook, partition_id_tensor)

        install_neuronx_cc_hook()
        self.jax = jax
        self.nc = nc
        self.n_cores = n_cores
        in_names, out_names, out_avals = [], [], []
        for alloc in nc.m.functions[0].allocations:
            if not isinstance(alloc, mybir.MemoryLocationSet):
                continue
            name = alloc.memorylocations[0].name
            if alloc.kind == "ExternalInput":
                if name != "partition_id":
                    in_names.append(name)
            elif alloc.kind == "ExternalOutput":
                out_names.append(name)
                out_avals.append(jax.core.ShapedArray(
                    tuple(alloc.tensor_shape), mybir.dt.np(alloc.dtype)))
        self.in_names = in_names
        self.out_names = out_names
        self.out_avals = out_avals
        all_in = in_names + out_names + ["partition_id"]
        n_ops = len(in_names) + len(out_names)

        def _body(*args):
            outs = _bass_exec_p.bind(
                *args, partition_id_tensor(),
                out_avals=tuple(out_avals),
                in_names=tuple(all_in),
                out_names=tuple(out_names),
                lowering_input_output_aliases=(),
                sim_require_finite=True,
                sim_require_nnan=True,
                nc=nc,
            )
            return tuple(outs)

        devices = jax.devices()[:n_cores]
        self.mesh = Mesh(np.asarray(devices), ("core",))
        spec = PartitionSpec("core")
        self.sharding = NamedSharding(self.mesh, spec)
        self.fn = jax.jit(
            shard_map(_body, mesh=self.mesh, in_specs=(spec,) * n_ops,
                      out_specs=(spec,) * len(out_names), check_rep=False),
            keep_unused=True)
        # device-resident zeros, reused every call (read-only operand)
        self.zero_out = [
            jax.device_put(
                np.zeros((n_cores * av.shape[0], *av.shape[1:]), av.dtype),
                self.sharding)
            for av in out_avals
        ]

    def __call__(self, in_maps):
        n = self.n_cores
        concat_in = [
            np.concatenate([np.asarray(in_maps[c][name]) for c in range(n)],
                           axis=0)
            for name in self.in_names
        ]
        outs = self.fn(*concat_in, *self.zero_out)
        out = np.asarray(outs[0]).reshape(n, *self.out_avals[0].shape)
        return [{self.out_names[0]: out[c]} for c in range(n)]


_RUNNER = None


def _get_runner():
    global _RUNNER
    if _RUNNER is None:
        _RUNNER = _Runner(_get_nc())
    return _RUNNER


def kernel(x, W_attn, b_attn, W_proj, b_proj):
    in_maps = _make_in_maps(x, W_attn, b_attn, W_proj)
    try:
        results = _get_runner()(in_maps)
    except Exception:
        res = run_bass_kernel_spmd(_get_nc(), in_maps, list(range(8)))
        results = res.results
    return _gather(results, b_proj)
